# revision 1
# baseline (speedup 1.0000x reference)
"""AttnReweight kernel for Trainium2 (8 NeuronCores, SPMD data parallel).

Semantics (matching the reference):
    c = max(attn); a = exp(attn - c)
    pj[b,s,h,w,k] = sum_t sims[b,hj,wj,t] * (sinds[b,hj,wj,t] == sinds[b,h,w,s])
                    where (hj,wj) = clamped 3x3 neighbor k of (h,w)
    m = a[b,d,h,w,k] * pj[b,s,h,w,k]
    out[b,d,s,h,w,k] = m / (1e-10 + sum_k m)

Sharding: core = b*4 + q handles image b, rows [48q, 48q+48), all heads d and
all superpixel slots s.  Row-shifted (dh) and col-padded (dw) neighbor copies
are materialized host-side so the device program is identical on every core
(pure SPMD, no collectives) and every compute op runs at partition base 0.

On-chip layout: 96 partitions = 2 w-halves x 48 rows; free dim = w x slot.
"""

import numpy as np

B, HD, H, W, K, NSP = 2, 8, 192, 192, 9, 9
NCORES = 8
ROWS = 48            # image rows per core
G = 2                # w segments per core
WSEG = 96            # interior w positions per segment
WSPAN = WSEG + 2     # with w halo
PI = G * ROWS        # 96 partitions
FR = WSPAN * 9       # 882 free elements for padded (w, slot) tiles
FI = WSEG * K        # 864 interior free elements (w, k)
EPS = 1e-10
OFFS = [(dh, dw) for dh in (-1, 0, 1) for dw in (-1, 0, 1)]

_compiled = None


def _build():
    from contextlib import ExitStack

    import concourse.bacc as bacc
    import concourse.tile as tile
    from concourse import mybir

    f32 = mybir.dt.float32
    Alu = mybir.AluOpType

    nc = bacc.Bacc(
        "TRN2",
        target_bir_lowering=False,
        debug=False,
        enable_asserts=True,
        num_devices=NCORES,
    )

    sind_d = nc.dram_tensor("sind3", [3, PI, FR], f32, kind="ExternalInput").ap()
    sims_d = nc.dram_tensor("sims3", [3, PI, FR], f32, kind="ExternalInput").ap()
    attn_d = nc.dram_tensor("attn_pad", [HD, PI, FI], f32, kind="ExternalInput").ap()
    negc_d = nc.dram_tensor("negc", [128, 1], f32, kind="ExternalInput").ap()
    eps_d = nc.dram_tensor("epsv", [128, 1], f32, kind="ExternalInput").ap()
    out_d = nc.dram_tensor(
        "out", [HD, NSP, ROWS, W, K], f32, kind="ExternalOutput"
    ).ap()

    with tile.TileContext(nc) as tc, ExitStack() as ctx:
        const = ctx.enter_context(tc.tile_pool(name="const", bufs=1))
        work = ctx.enter_context(tc.tile_pool(name="work", bufs=2))
        scr = ctx.enter_context(tc.tile_pool(name="scr", bufs=2))
        outp = ctx.enter_context(tc.tile_pool(name="outp", bufs=2))

        sind_t = [const.tile([PI, FR], f32, name=f"sind{i}") for i in range(3)]
        sims_t = [const.tile([PI, FR], f32, name=f"sims{i}") for i in range(3)]
        for i in range(3):
            nc.sync.dma_start(sind_t[i][:], sind_d[i])
            nc.sync.dma_start(sims_t[i][:], sims_d[i])
        negc_t = const.tile([128, 1], f32)
        nc.sync.dma_start(negc_t[:], negc_d)
        eps_t = const.tile([128, 1], f32)
        nc.sync.dma_start(eps_t[:], eps_d)
        # pj layout per partition: (wl 96, s 9, k 9)
        pj_t = const.tile([PI, WSEG * NSP * K], f32)

        s3 = [t[:].rearrange("p (w s) -> p w s", s=NSP) for t in sind_t]
        w3 = [t[:].rearrange("p (w s) -> p w s", s=NSP) for t in sims_t]
        pj4 = pj_t[:].rearrange("p (w s k) -> p w s k", s=NSP, k=K)

        si = s3[1][:, 1:97, :]  # queries (dh=0, interior w): [96, 96, 9]

        # ---- match: pj[., wl, s, k] = sum_t wjt * (sjt == si) ----
        # Slots are processed in groups of 4 (then 4, then 1): the eq and
        # mult run as one wide op over (t, w, s) and the 4 slot-planes are
        # summed with an in-place pairwise tree, amortizing per-op overhead.
        FQ1 = WSEG * NSP  # 864
        for ki, (dh, dw) in enumerate(OFFS):
            pjv = pj4[:, :, :, ki : ki + 1].squeeze(3)  # [96, 96, 9]
            si4 = si.unsqueeze(1).broadcast_to([PI, 4, WSEG, NSP])
            for t0 in (0, 4):
                sj4 = (
                    s3[dh + 1][:, 1 + dw : 97 + dw, t0 : t0 + 4]
                    .transpose([0, 2, 1])
                    .unsqueeze(3)
                    .broadcast_to([PI, 4, WSEG, NSP])
                )
                wj4 = (
                    w3[dh + 1][:, 1 + dw : 97 + dw, t0 : t0 + 4]
                    .transpose([0, 2, 1])
                    .unsqueeze(3)
                    .broadcast_to([PI, 4, WSEG, NSP])
                )
                eq = scr.tile([PI, 4 * FQ1], f32, tag="eq", bufs=1)
                e4 = eq[:].rearrange("p (t w s) -> p t w s", w=WSEG, s=NSP)
                nc.vector.tensor_tensor(e4, si4, sj4, Alu.is_equal)
                em = scr.tile([PI, 4 * FQ1], f32, tag="em", bufs=1)
                m4 = em[:].rearrange("p (t w s) -> p t w s", w=WSEG, s=NSP)
                nc.vector.tensor_tensor(m4, e4, wj4, Alu.mult)
                # pairwise tree: em[0:2] += em[2:4]; then slot0+slot1
                nc.vector.tensor_tensor(
                    em[:, 0 : 2 * FQ1], em[:, 0 : 2 * FQ1],
                    em[:, 2 * FQ1 : 4 * FQ1], Alu.add,
                )
                h0v = em[:, 0:FQ1].rearrange("p (w s) -> p w s", s=NSP)
                h1v = em[:, FQ1 : 2 * FQ1].rearrange("p (w s) -> p w s", s=NSP)
                if t0 == 0:
                    nc.vector.tensor_tensor(pjv, h0v, h1v, Alu.add)
                else:
                    nc.vector.tensor_tensor(h0v, h0v, h1v, Alu.add)
                    nc.vector.tensor_tensor(pjv, pjv, h0v, Alu.add)
            # final slot t=8
            sjt = s3[dh + 1][:, 1 + dw : 97 + dw, 8:9].broadcast_to(
                [PI, WSEG, NSP]
            )
            wjt = w3[dh + 1][:, 1 + dw : 97 + dw, 8:9].broadcast_to(
                [PI, WSEG, NSP]
            )
            eq1 = scr.tile([PI, FQ1], f32, tag="eq1")
            e3 = eq1[:].rearrange("p (w s) -> p w s", s=NSP)
            nc.vector.tensor_tensor(e3, si, sjt, Alu.is_equal)
            em1 = scr.tile([PI, FQ1], f32, tag="em1")
            m3 = em1[:].rearrange("p (w s) -> p w s", s=NSP)
            nc.vector.tensor_tensor(m3, e3, wjt, Alu.mult)
            nc.vector.tensor_tensor(pjv, pjv, m3, Alu.add)

        # ---- per-head normalize and store ----
        for d in range(HD):
            a_t = work.tile([PI, FI], f32, tag="a")
            nc.sync.dma_start(a_t[:], attn_d[d])
            ae_t = work.tile([PI, FI], f32, tag="ae")
            nc.scalar.activation(
                ae_t[:],
                a_t[:],
                mybir.ActivationFunctionType.Exp,
                bias=negc_t[0:PI, :],
                scale=1.0,
            )
            a_int = ae_t[:].rearrange("p (w k) -> p w k", k=K)

            outd = outp.tile([PI, NSP * WSEG * K], f32, tag="outd")  # (s, wl, k)
            den_t = scr.tile([PI, NSP * WSEG], f32, tag="den")  # (s, wl)
            rec_t = scr.tile([PI, NSP * WSEG], f32, tag="rec")

            # m = a * pj in 4-slot batches (s-groups 4+4+1)
            a4 = a_int.unsqueeze(1).broadcast_to([PI, 4, WSEG, K])
            for s0 in (0, 4):
                mv4 = outd[:, FI * s0 : FI * (s0 + 4)].rearrange(
                    "p (s w k) -> p s w k", w=WSEG, k=K
                )
                pj4s = pj4[:, :, s0 : s0 + 4, :].transpose([0, 2, 1, 3])
                nc.vector.tensor_tensor(mv4, a4, pj4s, Alu.mult)
            mv1 = outd[:, FI * 8 : FI * 9].rearrange("p (w k) -> p w k", k=K)
            nc.vector.tensor_tensor(
                mv1, a_int, pj4[:, :, 8:9, :].squeeze(2), Alu.mult
            )
            # den = sum_k m, one batched reduce over (s, w)
            nc.vector.tensor_reduce(
                den_t[:],
                outd[:].rearrange("p (x k) -> p x k", k=K),
                axis=mybir.AxisListType.X,
                op=Alu.add,
            )
            # rec = 1 / (den + eps)
            nc.vector.tensor_scalar(
                den_t[:], den_t[:], eps_t[0:PI, :], None, Alu.add
            )
            nc.vector.reciprocal_approx_fast(rec_t[:], den_t[:])
            # out = m * rec in 4-slot batches
            for s0 in (0, 4):
                mv4 = outd[:, FI * s0 : FI * (s0 + 4)].rearrange(
                    "p (s w k) -> p s w k", w=WSEG, k=K
                )
                rb4 = (
                    rec_t[:, WSEG * s0 : WSEG * (s0 + 4)]
                    .rearrange("p (s w) -> p s w", w=WSEG)
                    .unsqueeze(3)
                    .broadcast_to([PI, 4, WSEG, K])
                )
                nc.vector.tensor_tensor(mv4, mv4, rb4, Alu.mult)
            rb1 = (
                rec_t[:, WSEG * 8 : WSEG * 9]
                .unsqueeze(2)
                .broadcast_to([PI, WSEG, K])
            )
            nc.vector.tensor_tensor(mv1, mv1, rb1, Alu.mult)

            for g in range(G):
                src = outd[ROWS * g : ROWS * (g + 1), :].rearrange(
                    "p (s w k) -> p s w k", s=NSP, k=K
                )
                dst = out_d[d, :, :, WSEG * g : WSEG * (g + 1), :].transpose(
                    [1, 0, 2, 3]
                )  # [48, 9, 96, 9]
                nc.sync.dma_start(dst, src)

    nc.compile()
    return nc


def _get_compiled():
    global _compiled
    if _compiled is None:
        _compiled = _build()
    return _compiled


def _prep_core(attn, sims, sinds, negc, epsv, core):
    b, q = core // 4, core % 4
    h0 = q * ROWS
    cols = np.clip(np.arange(-1, W + 1), 0, W - 1)

    def pad3(x):  # x: [H, W, 9] -> [3, PI, FR]  (dh-shifted, w-padded copies)
        out = np.empty((3, PI, FR), np.float32)
        for i, dh in enumerate((-1, 0, 1)):
            rows = np.clip(np.arange(h0, h0 + ROWS) + dh, 0, H - 1)
            xp = x[rows][:, cols, :]  # [48, 194, 9]
            segs = [xp[:, WSEG * g : WSEG * g + WSPAN, :] for g in range(G)]
            out[i] = np.concatenate(segs, axis=0).reshape(PI, FR)
        return out

    sind3 = pad3(sinds[b])
    sims3 = pad3(sims[b])
    ap = attn[b][:, h0 : h0 + ROWS]  # [HD, 48, 192, 9]
    segs = [ap[:, :, WSEG * g : WSEG * (g + 1), :] for g in range(G)]
    attn_pad = np.concatenate(segs, axis=1).reshape(HD, PI, FI).astype(np.float32)
    return {
        "sind3": np.ascontiguousarray(sind3),
        "sims3": np.ascontiguousarray(sims3),
        "attn_pad": np.ascontiguousarray(attn_pad),
        "negc": negc,
        "epsv": epsv,
    }


def kernel(attn, sims, sinds, _trace=False):
    attn = np.asarray(attn)
    sims = np.asarray(sims)
    sinds = np.asarray(sinds)

    from concourse import bass_utils

    nc = _get_compiled()

    c = float(np.max(attn))
    negc = np.full((128, 1), -c, dtype=np.float32)
    epsv = np.full((128, 1), EPS, dtype=np.float32)
    in_maps = [
        _prep_core(attn, sims, sinds, negc, epsv, core) for core in range(NCORES)
    ]
    res = bass_utils.run_bass_kernel_spmd(
        nc, in_maps, core_ids=list(range(NCORES)), trace=_trace
    )
    out = np.empty((B, HD, NSP, H, W, K), dtype=np.float32)
    for core in range(NCORES):
        b, q = core // 4, core % 4
        out[b, :, :, ROWS * q : ROWS * (q + 1)] = res.results[core]["out"]
    if _trace:
        return out, res
    return out



# revision 13
# speedup vs baseline: 1.9348x; 1.9348x over previous
"""AttnReweight kernel for Trainium2 (8 NeuronCores, SPMD data parallel).

Semantics (matching the reference):
    c = max(attn); a = exp(attn - c)
    pj[b,s,h,w,k] = sum_t sims[b,hj,wj,t] * (sinds[b,hj,wj,t] == sinds[b,h,w,s])
                    where (hj,wj) = clamped 3x3 neighbor k of (h,w)
    m = a[b,d,h,w,k] * pj[b,s,h,w,k]
    out[b,d,s,h,w,k] = m / (1e-10 + sum_k m)

Sharding: core = b*4 + q handles image b, rows [48q, 48q+48), all heads and
slots.  Pixels are flattened row-major (x = 9216 per core) and chopped into
128 partitions x 72 pixels.  All 9 (dh,dw) neighbor shifts are materialized
host-side as clamped copies in slot-major bf16 layout, so every device-side
access is a static packed slice (eligible for the DVE 2-byte 2x mode).

Engine split per core: DVE does eq/mult/tree-sum (match) and the
mult/den/normalize chain (head); Act does exp and the rec->rec9 k-replication;
output is written bf16 and upcast to f32 on the host.
"""

import numpy as np

B, HD, H, W, K, NSP = 2, 8, 192, 192, 9, 9
NCORES = 8
ROWS = 48              # image rows per core
X = ROWS * W           # 9216 flattened pixels per core
P = 128                # partitions
XL = X // P            # 72 pixels per partition
FS = NSP * XL          # 648 = (s, xl) block
FK = XL * K            # 648 = (xl, k) block
FM = NSP * XL * K      # 5832 = (s, xl, k) block
EPS = 1e-10
OFFS = [(dh, dw) for dh in (-1, 0, 1) for dw in (-1, 0, 1)]
CENTER = OFFS.index((0, 0))

_compiled = None


def _build():
    from contextlib import ExitStack

    import concourse.bacc as bacc
    import concourse.tile as tile
    from concourse import mybir

    f32 = mybir.dt.float32
    bf16 = mybir.dt.bfloat16
    f16 = mybir.dt.float16
    Alu = mybir.AluOpType
    Act = mybir.ActivationFunctionType

    nc = bacc.Bacc(
        "TRN2",
        target_bir_lowering=False,
        debug=False,
        enable_asserts=True,
        num_devices=NCORES,
    )

    # 9 shifted copies, slot-major: [o, p, (t, xl)]; sims pre-scaled by 1024
    # (fp16 keeps the whole match pipeline in normal range; the scale cancels
    # in out = m * rec except through eps, which is scaled to match).
    sind_d = nc.dram_tensor("sind9", [9, P, NSP * XL], f16, kind="ExternalInput").ap()
    sims_d = nc.dram_tensor("sims9", [9, P, NSP * XL], f16, kind="ExternalInput").ap()
    attn_d = nc.dram_tensor("attn_x", [HD, P, FK], f32, kind="ExternalInput").ap()
    negc_d = nc.dram_tensor("negc", [P, 1], f32, kind="ExternalInput").ap()
    out_d = nc.dram_tensor("out", [HD, NSP, X, K], bf16, kind="ExternalOutput").ap()

    with tile.TileContext(nc) as tc, ExitStack() as ctx, nc.allow_low_precision(
        reason="bf16 pipeline validated against 2e-2 harness tolerance"
    ):
        const = ctx.enter_context(tc.tile_pool(name="const", bufs=1))
        scr = ctx.enter_context(tc.tile_pool(name="scr", bufs=1))
        work = ctx.enter_context(tc.tile_pool(name="work", bufs=2))

        sind_t = [const.tile([P, NSP * XL], f16, name=f"sind{i}") for i in range(9)]
        sims_t = [const.tile([P, NSP * XL], f16, name=f"sims{i}") for i in range(9)]
        for i in range(9):
            nc.sync.dma_start(sind_t[i][:], sind_d[i])
            nc.sync.dma_start(sims_t[i][:], sims_d[i])
        negc_t = const.tile([P, 1], f32)
        nc.sync.dma_start(negc_t[:], negc_d)

        eq_t = const.tile([P, 9 * FS], f16)     # (t, s, xl)
        em_t = const.tile([P, 9 * FS], f16)     # (t, s, xl), tree runs in-place
        pjc_t = const.tile([P, 9 * FS], f16)    # (k, s, xl) compact pj (x1024)
        pj9_t = const.tile([P, FM], f16)        # (s, xl, k) k-replicated view

        # query = center copy viewed (t->s), broadcast over t
        qry = (
            sind_t[CENTER][:]
            .rearrange("p (s x) -> p s x", s=NSP)
            .unsqueeze(1)
            .broadcast_to([P, 9, NSP, XL])
        )
        eq4 = eq_t[:].rearrange("p (t s x) -> p t s x", t=9, s=NSP)
        em4 = em_t[:].rearrange("p (t s x) -> p t s x", t=9, s=NSP)

        # ---- match: pjc[k, s, xl] = sum_t sims_k[t, xl] * eq ----
        for o in range(9):
            nbr = (
                sind_t[o][:]
                .rearrange("p (t x) -> p t x", t=9)
                .unsqueeze(2)
                .broadcast_to([P, 9, NSP, XL])
            )
            wgt = (
                sims_t[o][:]
                .rearrange("p (t x) -> p t x", t=9)
                .unsqueeze(2)
                .broadcast_to([P, 9, NSP, XL])
            )
            nc.vector.tensor_tensor(eq4, nbr, qry, Alu.is_equal)
            nc.vector.tensor_tensor(em4, eq4, wgt, Alu.mult)
            # tree-sum over t: 9 = (0:4)+(4:8), pairwise, + t=8
            nc.vector.tensor_tensor(
                em_t[:, 0 : 4 * FS], em_t[:, 0 : 4 * FS],
                em_t[:, 4 * FS : 8 * FS], Alu.add,
            )
            nc.vector.tensor_tensor(
                em_t[:, 0 : 2 * FS], em_t[:, 0 : 2 * FS],
                em_t[:, 2 * FS : 4 * FS], Alu.add,
            )
            nc.vector.tensor_tensor(
                em_t[:, 0:FS], em_t[:, 0:FS], em_t[:, FS : 2 * FS], Alu.add
            )
            nc.vector.tensor_tensor(
                pjc_t[:, o * FS : (o + 1) * FS],
                em_t[:, 0:FS],
                em_t[:, 8 * FS : 9 * FS],
                Alu.add,
            )
        # transpose-replicate (k, s, xl) -> (s, xl, k) in one 2x tensor_scalar
        pjc_v = (
            pjc_t[:]
            .rearrange("p (k s x) -> p k s x", k=9, s=NSP)
            .transpose([0, 2, 3, 1])  # (s, xl, k)
        )
        pj9_v = pj9_t[:].rearrange("p (s x k) -> p s x k", s=NSP, k=K)
        nc.vector.tensor_scalar(pj9_v, pjc_v, 1.0, None, Alu.mult)

        # ---- per-head normalize and store ----
        den_t = scr.tile([P, FS], f32)
        rec_t = scr.tile([P, FS], f32)
        t4_t = scr.tile([P, FS * 4], bf16)
        t2_t = scr.tile([P, FS * 2], f32)

        for d in range(HD):
            a_t = work.tile([P, FK], f32, tag="a")
            nc.sync.dma_start(a_t[:], attn_d[d])
            ae_t = work.tile([P, FK], f16, tag="ae")
            nc.scalar.activation(
                ae_t[:], a_t[:], Act.Exp, bias=negc_t[0:P, :], scale=1.0
            )

            m_t = work.tile([P, FM], bf16, tag="m")
            ae_v = (
                ae_t[:]
                .rearrange("p (x k) -> p x k", k=K)
                .unsqueeze(1)
                .broadcast_to([P, NSP, XL, K])
            )
            m_v = m_t[:].rearrange("p (s x k) -> p s x k", s=NSP, k=K)
            nc.vector.tensor_tensor(m_v, ae_v, pj9_v, Alu.mult)

            # den[s,xl] = sum_k m: k-slice tree
            m3 = m_t[:].rearrange("p (sx k) -> p sx k", k=K)
            t4v = t4_t[:].rearrange("p (sx k) -> p sx k", k=4)
            t2v = t2_t[:].rearrange("p (sx k) -> p sx k", k=2)
            nc.vector.tensor_tensor(t4v, m3[:, :, 0:4], m3[:, :, 4:8], Alu.add)
            nc.vector.tensor_tensor(t2v, t4v[:, :, 0:2], t4v[:, :, 2:4], Alu.add)
            nc.vector.tensor_tensor(
                den_t[:], t2v[:, :, 0:1].squeeze(2), t2v[:, :, 1:2].squeeze(2), Alu.add
            )
            nc.vector.tensor_tensor(
                den_t[:], den_t[:], m3[:, :, 8:9].squeeze(2), Alu.add
            )
            # rec = 1/(den + eps); eps scaled to match sims x1024 and ae x16
            nc.vector.tensor_scalar(den_t[:], den_t[:], EPS * 16384.0, None, Alu.add)
            nc.vector.reciprocal_approx_fast(rec_t[:], den_t[:])

            # rec9[s, xl, k] = rec[s, xl] replicated over k (Act engine)
            rec9_t = work.tile([P, FM], bf16, tag="rec9")
            rec9_v = rec9_t[:].rearrange("p (s x k) -> p s x k", s=NSP, k=K)
            rec_v = (
                rec_t[:]
                .rearrange("p (s x) -> p s x", s=NSP)
                .unsqueeze(3)
                .broadcast_to([P, NSP, XL, K])
            )
            nc.scalar.activation(rec9_v, rec_v, Act.Copy)

            nc.vector.tensor_tensor(m_t[:], m_t[:], rec9_t[:], Alu.mult)

            dst = out_d[d].rearrange("s (pp x) k -> pp s x k", pp=P)
            src = m_t[:].rearrange("p (s x k) -> p s x k", s=NSP, k=K)
            nc.sync.dma_start(dst, src)

    nc.compile()
    return nc


def _get_compiled():
    global _compiled
    if _compiled is None:
        _compiled = _build()
    return _compiled


def _prep_core(attn, sims, sinds, negc, core, bf16):
    b, q = core // 4, core % 4
    h0 = q * ROWS
    rows = np.arange(h0, h0 + ROWS)
    cols = np.arange(W)

    def shifted9(x, dt):  # x: [H, W, 9] -> [9, P, 9*XL] slot-major shifted copies
        out = np.empty((9, P, NSP * XL), dt)
        for i, (dh, dw) in enumerate(OFFS):
            r = np.clip(rows + dh, 0, H - 1)
            c = np.clip(cols + dw, 0, W - 1)
            v = x[r][:, c, :].reshape(X, NSP)          # [9216, 9]
            v = v.reshape(P, XL, NSP).transpose(0, 2, 1)  # [128, 9(t), 72]
            out[i] = v.reshape(P, NSP * XL)
        return out

    sind9 = shifted9(sinds[b].astype(np.float32), np.float16)
    sims9 = shifted9(sims[b] * 1024.0, np.float16)
    attn_x = np.ascontiguousarray(
        attn[b][:, h0 : h0 + ROWS].reshape(HD, P, FK), dtype=np.float32
    )
    return {
        "sind9": np.ascontiguousarray(sind9),
        "sims9": np.ascontiguousarray(sims9),
        "attn_x": attn_x,
        "negc": negc,
    }


def kernel(attn, sims, sinds, _trace=False):
    import ml_dtypes

    attn = np.asarray(attn)
    sims = np.asarray(sims)
    sinds = np.asarray(sinds)
    bf16 = ml_dtypes.bfloat16

    from concourse import bass_utils

    nc = _get_compiled()

    # exp bias: -c plus ln(16) so ae = 16*exp(attn-c) stays in fp16 normal range
    c = float(np.max(attn))
    negc = np.full((P, 1), -c + float(np.log(16.0)), dtype=np.float32)
    in_maps = [
        _prep_core(attn, sims, sinds, negc, core, bf16) for core in range(NCORES)
    ]
    res = bass_utils.run_bass_kernel_spmd(
        nc, in_maps, core_ids=list(range(NCORES)), trace=_trace
    )
    out = np.empty((B, HD, NSP, H, W, K), dtype=np.float32)
    for core in range(NCORES):
        b, q = core // 4, core % 4
        o = np.asarray(res.results[core]["out"]).astype(np.float32)
        out[b, :, :, ROWS * q : ROWS * (q + 1)] = o.reshape(HD, NSP, ROWS, W, K)
    if _trace:
        return out, res
    return out
